# revision 31
# baseline (speedup 1.0000x reference)
"""Trainium2 Bass kernel for nn_MultiHeadAttention_67379446939752.

Per-token multi-head attention:
  Q = q @ Wq.T + bq ; K,V likewise        [B,S,D] -> [B,S,H,HD]
  score[t,h,g] = sum_d Q[t,h,d] K[t,g,d]  (per-token HxH gram, no seq mixing)
  attn[t] = softmax(score[t]) @ V[t]      -> [B,S,D]
  out = attn @ Wo.T + bo

v2 strategy (wall-clock per call is dominated by host<->device transfer, so
minimize wire bytes first, then keep HW exec near the PE roofline):
  - Data-parallel over the 16384 tokens across 8 NeuronCores (2048/core).
  - fp16 wire format for activations, weights and output (max rel err vs
    fp64 reference ~2.4e-3, an 8x margin under the 2e-2 gate).
  - Weights are sharded 8-ways on the wire (256 rows each) and AllGathered
    on-device over NeuronLink: 536MB of replicated weight traffic -> 33.5MB.
  - Natural [T,D]/[D,D] row-major layouts on the wire; the contraction-dim
    transposes happen on-device as cheap PE transpose ops (fp16: 128 cyc per
    128x128 tile), so the host does no big transposes.
  - All big matmuls in fp16 operands (full PE rate, fp32 PSUM accumulate).
  - The per-token 16x16 attention runs 8 tokens at a time as 128x128 fp16
    matmuls whose cross-token blocks are pushed to -1024 in PSUM by a rank-8
    mask matmul; exp() then zeroes them exactly (block-diagonal softmax with
    no DVE masking pass).
  - Attention and the output projection are fused per 256-token chunk (the
    attn result never round-trips through DRAM).
"""
import sys
sys.path.insert(0, "/opt/trn_rl_repo")
import numpy as np
import concourse.bass as bass
import concourse.mybir as mybir
import concourse.bacc as bacc
import concourse.tile as tile
from concourse.bass_utils import run_bass_kernel_spmd

B, S, D, H, HD = 4, 4096, 2048, 16, 128
NCORES = 8
T_FULL = B * S
F16, F32 = mybir.dt.float16, mybir.dt.float32
KT = D // 128            # contraction tiles
SHIFT = 25.0             # constant softmax shift (softmax-invariant)
NEG = 1024.0             # additive mask magnitude for cross-token blocks
TA = 256                 # token chunk
Exp = mybir.ActivationFunctionType.Exp


def mask_consts():
    # u8[r,(t,h)] = 1 if t==r ; v8[r,(t',g)] = -NEG*(1 - (t'==r))
    u = np.zeros((8, 128), np.float16)
    for r in range(8):
        u[r, r * 16:(r + 1) * 16] = 1.0
    v = np.full((8, 128), -NEG, np.float16)
    for r in range(8):
        v[r, r * 16:(r + 1) * 16] = 0.0
    return u, v


def build(T, ncores=NCORES, shared_gather=True):
    NCH = T // TA            # chunks
    NBK = TA // 8            # 8-token blocks per chunk
    NTB = TA // 128          # 128-token row tiles per chunk
    DS_ = D // ncores        # weight shard rows
    nc = bacc.Bacc(None, target_bir_lowering=False, num_devices=ncores)
    xq = nc.dram_tensor("xq", [T, D], F16, kind="ExternalInput")
    xk = nc.dram_tensor("xk", [T, D], F16, kind="ExternalInput")
    xv = nc.dram_tensor("xv", [T, D], F16, kind="ExternalInput")
    wqs = nc.dram_tensor("wqs", [DS_, D], F16, kind="ExternalInput")
    wks = nc.dram_tensor("wks", [DS_, D], F16, kind="ExternalInput")
    wvs = nc.dram_tensor("wvs", [DS_, D], F16, kind="ExternalInput")
    wos = nc.dram_tensor("wos", [DS_, D], F16, kind="ExternalInput")
    bq2 = nc.dram_tensor("bq2", [128, H], F32, kind="ExternalInput")
    bk2 = nc.dram_tensor("bk2", [128, H], F32, kind="ExternalInput")
    bv2 = nc.dram_tensor("bv2", [128, H], F32, kind="ExternalInput")
    bo_row = nc.dram_tensor("bo_row", [1, D], F16, kind="ExternalInput")
    ones_row = nc.dram_tensor("ones_row", [1, 128], F16, kind="ExternalInput")
    out_d = nc.dram_tensor("out", [T, D], F16, kind="ExternalOutput")

    u8_np, v8_np = mask_consts()
    u8_d = nc.inline_tensor(u8_np, "u8c")
    v8_d = nc.inline_tensor(v8_np, "v8c")
    id_d = nc.inline_tensor(np.eye(128, dtype=np.float16), "id128")

    with tile.TileContext(nc) as tc:
        with (
            tc.tile_pool(name="dram", bufs=1, space="DRAM") as dpool,
            tc.tile_pool(name="const", bufs=1) as cpool,
        ):
            u8 = cpool.tile([8, 128], F16, tag="u8")
            v8 = cpool.tile([8, 128], F16, tag="v8")
            identF = cpool.tile([128, 128], F16, tag="identF")
            nc.sync.dma_start(u8[:], u8_d[:])
            nc.sync.dma_start(v8[:], v8_d[:])
            nc.sync.dma_start(identF[:], id_d[:])
            biasq = cpool.tile([128, H], F32, tag="bq")
            biask = cpool.tile([128, H], F32, tag="bk")
            biasv = cpool.tile([128, H], F32, tag="bvt")
            bor = cpool.tile([1, D], F16, tag="bo")
            onesr = cpool.tile([1, 128], F16, tag="ones")
            nc.sync.dma_start(biasq[:], bq2[:])
            nc.sync.dma_start(biask[:], bk2[:])
            nc.sync.dma_start(biasv[:], bv2[:])
            nc.sync.dma_start(bor[:], bo_row[:])
            nc.sync.dma_start(onesr[:], ones_row[:])
            shiftc = cpool.tile([128, 1], F32, tag="shiftc")
            nc.vector.memset(shiftc[:], -SHIFT)

            # ---- weight shards: transpose locally (no gather dep), then
            # AllGather pre-transposed shards straight into W^T layout ----
            # WgT[i] is [D(d), D(j)] fp16 = W^T; rank c's contribution lands in
            # columns [c*DS_, (c+1)*DS_) via the rank-major output AP.
            WgT = []
            NSB = DS_ // 128         # 128-row blocks per shard
            with (
                tc.tile_pool(name="shx", bufs=2) as shp,
                tc.tile_pool(name="sht", bufs=2) as stp0,
                tc.tile_pool(name="psSh", bufs=4, space="PSUM") as psh,
            ):
                for i, wsh in enumerate((wqs, wks, wvs, wos)):
                    # rank-major contiguous gather output: block c is
                    # W^T[:, c*DS_:(c+1)*DS_] as a [D, DS_] tile
                    gg = dpool.tile([ncores * D, DS_], F16, tag=f"WgT{i}",
                                    name=f"WgT{i}",
                                    addr_space="Shared" if shared_gather else "Local")
                    wtb = dpool.tile([D, DS_], F16, tag=f"wtb{i}", name=f"wtb{i}")
                    sh = shp.tile([128, NSB, D], F16, tag="sh")
                    nc.sync.dma_start(
                        sh[:], wsh[:].rearrange("(b p) d -> p b d", p=128))
                    shT = stp0.tile([128, KT, DS_], F16, tag="shT")
                    for b in range(NSB):
                        for dh in range(KT // 8):
                            pw = psh.tile([128, 8, 128], F16, tag="psSh")
                            for dl in range(8):
                                dt = dh * 8 + dl
                                nc.tensor.matmul(
                                    pw[:, dl, :], sh[:, b, dt * 128:(dt + 1) * 128],
                                    identF[:], is_transpose=True, skip_group_check=True)
                            nc.any.tensor_copy(
                                shT[:, dh * 8:(dh + 1) * 8, b * 128:(b + 1) * 128],
                                pw[:])
                    nc.sync.dma_start(
                        wtb[:].rearrange("(dt p) jl -> p dt jl", p=128), shT[:])
                    if ncores == 1:
                        nc.gpsimd.dma_start(gg[:], wtb[:])
                    else:
                        nc.gpsimd.collective_compute(
                            "AllGather", mybir.AluOpType.bypass,
                            replica_groups=[list(range(ncores))],
                            ins=[wtb[:]], outs=[gg[:]])
                    WgT.append(gg)

            # per-chunk QKV spill tiles (fine-grained cross-phase deps)
            QT_ds = [dpool.tile([128, TA * H], F16, tag=f"QTd{i}", name=f"QTd{i}") for i in range(NCH)]
            KT_ds = [dpool.tile([128, TA * H], F16, tag=f"KTd{i}", name=f"KTd{i}") for i in range(NCH)]
            VT_ds = [dpool.tile([128, TA * H], F16, tag=f"VTd{i}", name=f"VTd{i}") for i in range(NCH)]

            NBC = 512 // DS_         # rank blocks per 512-col quarter

            def load_wt_quarters(pool, wg, tag, eng):
                # gathered W^T DRAM [(c d), jl] rank-major -> 4 SBUF tiles
                # [128 d-part, dt, 512 j]; quarter q covers rank blocks
                # c in [q*NBC, (q+1)*NBC).  These DMAs wait on the AllGather,
                # so they get their own queue (eng) to avoid head-of-line
                # blocking the activation-path DMAs.
                parts = []
                for q in range(4):
                    wq_ = pool.tile([128, KT, 512], F16, tag=f"{tag}{q}",
                                    name=f"{tag}{q}")
                    for b in range(NBC):
                        c = q * NBC + b
                        eng.dma_start(
                            wq_[:, :, b * DS_:(b + 1) * DS_],
                            wg[c * D:(c + 1) * D, :].rearrange(
                                "(dt p) jl -> p dt jl", p=128))
                    parts.append(wq_)
                return parts

            # ---------------- Phase A: QKV projections ----------------
            for xin, wg, bias, spills in (
                (xq, WgT[0], biasq, QT_ds),
                (xk, WgT[1], biask, KT_ds),
                (xv, WgT[2], biasv, VT_ds),
            ):
                with (
                    tc.tile_pool(name="wt", bufs=1) as wtp,
                    tc.tile_pool(name="xb", bufs=2) as xbp,
                    tc.tile_pool(name="xt", bufs=2) as xtp,
                    tc.tile_pool(name="stA", bufs=2) as stp,
                    tc.tile_pool(name="psA", bufs=4, space="PSUM") as psA,
                    tc.tile_pool(name="psT", bufs=4, space="PSUM") as psT,
                ):
                    WT = load_wt_quarters(wtp, wg, "WT", nc.gpsimd)
                    for c in range(NCH):
                        xn = xbp.tile([128, NTB, D], F16, tag="xn")
                        nc.sync.dma_start(
                            xn[:], xin[c * TA:(c + 1) * TA, :].rearrange(
                                "(tb p) d -> p tb d", p=128))
                        xT = xtp.tile([128, KT, TA], F16, tag="xT")
                        for tb in range(NTB):
                            for kh in range(KT // 8):
                                px = psT.tile([128, 8, 128], F16, tag="psT")
                                for kl in range(8):
                                    kk = kh * 8 + kl
                                    nc.tensor.matmul(
                                        px[:, kl, :], xn[:, tb, kk * 128:(kk + 1) * 128],
                                        identF[:], is_transpose=True, skip_group_check=True)
                                nc.any.tensor_copy(
                                    xT[:, kh * 8:(kh + 1) * 8, tb * 128:(tb + 1) * 128],
                                    px[:])
                        stg = stp.tile([128, TA, H], F16, tag="stA")
                        for jg in range(4):
                            pss = [psA.tile([128, TA], F32, tag="psA",
                                            name=f"psA{jg}_{j}") for j in range(4)]
                            for kk in range(KT):
                                for jl in range(4):
                                    nc.tensor.matmul(
                                        pss[jl][:],
                                        WT[jg][:, kk, jl * 128:(jl + 1) * 128],
                                        xT[:, kk, :], start=(kk == 0), stop=(kk == KT - 1))
                            for jl in range(4):
                                jt = jg * 4 + jl
                                nc.any.tensor_scalar_add(stg[:, :, jt], pss[jl][:],
                                                         bias[:, jt:jt + 1])
                        nc.sync.dma_start(spills[c][:], stg[:].rearrange("p t h -> p (t h)"))

            # ------- Phase B+C fused: per-token attention + out proj -------
            with (
                tc.tile_pool(name="wo", bufs=1) as wop,
                tc.tile_pool(name="qk", bufs=2) as qkp,
                tc.tile_pool(name="vbp", bufs=2) as vbp,
                tc.tile_pool(name="attc", bufs=2) as atp,
                tc.tile_pool(name="eb", bufs=6) as ebp,
                tc.tile_pool(name="zb", bufs=8) as zbp,
                tc.tile_pool(name="stC", bufs=4) as stp2,
                tc.tile_pool(name="psS", bufs=2, space="PSUM") as psS,
                tc.tile_pool(name="psT2", bufs=2, space="PSUM") as psT2,
                tc.tile_pool(name="psA2", bufs=2, space="PSUM") as psA2,
                tc.tile_pool(name="psC", bufs=2, space="PSUM") as psC,
            ):
                WoT = load_wt_quarters(wop, WgT[3], "WoT", nc.gpsimd)
                NG = NBK // 4           # groups of 4 blocks (32 tokens)

                def load_chunk(c):
                    QTs = qkp.tile([128, TA, H], F16, tag="QTs")
                    KTs = qkp.tile([128, TA, H], F16, tag="KTs")
                    VTs = vbp.tile([128, TA, H], F16, tag="VTs")
                    nc.gpsimd.dma_start(QTs[:], QT_ds[c][:].rearrange("p (t h) -> p t h", h=H))
                    nc.gpsimd.dma_start(KTs[:], KT_ds[c][:].rearrange("p (t h) -> p t h", h=H))
                    nc.gpsimd.dma_start(VTs[:], VT_ds[c][:].rearrange("p (t h) -> p t h", h=H))
                    ATTc = atp.tile([128, H, TA], F16, tag="ATTc")
                    return {"QTs": QTs, "KTs": KTs, "VTs": VTs, "ATTc": ATTc}

                def issue_scores(st, c, g):
                    # scores for 4 blocks -> one packed PSUM bank
                    psb = psS.tile([128, 4, 128], F32, tag="psS", name=f"psb{c}_{g}")
                    for i in range(4):
                        sl = slice((g * 4 + i) * 8, (g * 4 + i + 1) * 8)
                        nc.tensor.matmul(
                            psb[:, i, :],
                            st["QTs"][:, sl, :].rearrange("p t h -> p (t h)"),
                            st["KTs"][:, sl, :].rearrange("p t h -> p (t h)"),
                            start=True, stop=False, skip_group_check=True)
                        nc.tensor.matmul(psb[:, i, :], u8[:], v8[:],
                                         start=False, stop=True, skip_group_check=True)
                    return psb

                def issue_attend(st, g, psb):
                    # softmax (ACT/DVE) then transposes + attn matmuls (PE)
                    E = ebp.tile([128, 4, 128], F32, tag="E")
                    Z4 = zbp.tile([128, 4], F32, tag="Z4")
                    for i in range(4):
                        nc.scalar.activation(E[:, i, :], psb[:, i, :], Exp,
                                             bias=shiftc[:], accum_out=Z4[:, i:i + 1])
                    R4 = zbp.tile([128, 4], F32, tag="R4")
                    nc.vector.reciprocal(R4[:], Z4[:])
                    Wb = ebp.tile([128, 4, 128], F16, tag="Wb")
                    for i in range(4):
                        nc.vector.tensor_scalar_mul(Wb[:, i, :], E[:, i, :],
                                                    R4[:, i:i + 1])
                    pt = psT2.tile([128, 8, 128], F16, tag="ps16")
                    for i in range(4):
                        sl = slice((g * 4 + i) * 8, (g * 4 + i + 1) * 8)
                        nc.tensor.matmul(pt[:, i, :], Wb[:, i, :], identF[:],
                                         is_transpose=True, skip_group_check=True)
                        nc.tensor.matmul(
                            pt[:, 4 + i, :],
                            st["VTs"][:, sl, :].rearrange("p t h -> p (t h)"), identF[:],
                            is_transpose=True, skip_group_check=True)
                    WVb = ebp.tile([128, 8, 128], F16, tag="WVb")
                    nc.any.tensor_copy(WVb[:], pt[:])
                    psa = psA2.tile([128, 4, 128], F32, tag="psA2")
                    for i in range(4):
                        nc.tensor.matmul(psa[:, i, :], WVb[:, 4 + i, :],
                                         WVb[:, i, :], start=True, stop=True,
                                         skip_group_check=True)
                    nc.any.tensor_copy(
                        st["ATTc"][:, :, g * 32:(g + 1) * 32].rearrange(
                            "p h (b t) -> p b t h", b=4),
                        psa[:].rearrange("p b (t h) -> p b t h", t=8))

                def issue_cpart(st, c):
                    # output projection for chunk c (attn stays in SBUF)
                    for tb in range(NTB):
                        for jc in range(D // 512):
                            ps = psC.tile([128, 512], F32, tag="psC")
                            for hh in range(KT):
                                nc.tensor.matmul(
                                    ps[:], st["ATTc"][:, hh, tb * 128:(tb + 1) * 128],
                                    WoT[jc][:, hh, :],
                                    start=(hh == 0), stop=False)
                            nc.tensor.matmul(ps[:], onesr[:], bor[:, jc * 512:(jc + 1) * 512],
                                             start=False, stop=True)
                            st_ = stp2.tile([128, 512], F16, tag="stC")
                            nc.any.tensor_copy(st_[:], ps[:])
                            nc.sync.dma_start(
                                out_d[c * TA + tb * 128:c * TA + (tb + 1) * 128,
                                      jc * 512:(jc + 1) * 512], st_[:])

                # cross-chunk pipeline: C-part of chunk c-1 issues right after
                # the first score group of chunk c, hiding softmax latency and
                # the chunk-boundary ATTc dependency under C's matmuls.
                prev_st = None
                for c in range(NCH):
                    st = load_chunk(c)
                    prevg = issue_scores(st, c, 0)
                    if prev_st is not None:
                        issue_cpart(prev_st, c - 1)
                    for g in range(1, NG):
                        cur = issue_scores(st, c, g)
                        issue_attend(st, g - 1, prevg)
                        prevg = cur
                    issue_attend(st, NG - 1, prevg)
                    prev_st = st
                issue_cpart(prev_st, NCH - 1)
    nc.compile()
    return nc


_cache = {}


def get_nc(T):
    if T not in _cache:
        _cache[T] = build(T)
    return _cache[T]


def make_in_maps(q, k, v, Wq, bq, Wk, bk, Wv, bv, Wo, bo, ncores=NCORES, T=None):
    f16, f32 = np.float16, np.float32
    q = np.asarray(q, f32).reshape(-1, D).astype(f16)
    k = np.asarray(k, f32).reshape(-1, D).astype(f16)
    v = np.asarray(v, f32).reshape(-1, D).astype(f16)
    if T is None:
        T = q.shape[0] // ncores
    DS_ = D // ncores
    W16 = [np.asarray(W, f32).astype(f16) for W in (Wq, Wk, Wv, Wo)]
    b2 = [np.ascontiguousarray(np.asarray(b, f32).reshape(H, 128).T)
          for b in (bq, bk, bv)]
    bo_row = np.asarray(bo, f32).astype(f16).reshape(1, D)
    ones = np.ones((1, 128), f16)
    maps = []
    for c in range(ncores):
        sl = slice(c * T, (c + 1) * T)
        ws = slice(c * DS_, (c + 1) * DS_)
        maps.append({
            "xq": q[sl], "xk": k[sl], "xv": v[sl],
            "wqs": W16[0][ws], "wks": W16[1][ws],
            "wvs": W16[2][ws], "wos": W16[3][ws],
            "bq2": b2[0], "bk2": b2[1], "bv2": b2[2],
            "bo_row": bo_row, "ones_row": ones,
        })
    return maps, T


def kernel(q, k, v, Wq, bq, Wk, bk, Wv, bv, Wo, bo):
    maps, T = make_in_maps(q, k, v, Wq, bq, Wk, bk, Wv, bv, Wo, bo)
    nc = get_nc(T)
    res = run_bass_kernel_spmd(nc, maps, list(range(NCORES)))
    out = np.concatenate([np.asarray(r["out"]) for r in res.results], axis=0)
    return out.reshape(B, S, D).astype(np.float32)


# revision 38
# speedup vs baseline: 1.0206x; 1.0206x over previous
"""Trainium2 Bass kernel for nn_MultiHeadAttention_67379446939752.

Per-token multi-head attention:
  Q = q @ Wq.T + bq ; K,V likewise        [B,S,D] -> [B,S,H,HD]
  score[t,h,g] = sum_d Q[t,h,d] K[t,g,d]  (per-token HxH gram, no seq mixing)
  attn[t] = softmax(score[t]) @ V[t]      -> [B,S,D]
  out = attn @ Wo.T + bo

v2 strategy (wall-clock per call is dominated by host<->device transfer, so
minimize wire bytes first, then keep HW exec near the PE roofline):
  - Data-parallel over the 16384 tokens across 8 NeuronCores (2048/core).
  - fp16 wire format for activations, weights and output (max rel err vs
    fp64 reference ~2.4e-3, an 8x margin under the 2e-2 gate).
  - Weights are sharded 8-ways on the wire (256 rows each) and AllGathered
    on-device over NeuronLink: 536MB of replicated weight traffic -> 33.5MB.
  - Natural [T,D]/[D,D] row-major layouts on the wire; the contraction-dim
    transposes happen on-device as cheap PE transpose ops (fp16: 128 cyc per
    128x128 tile), so the host does no big transposes.
  - All big matmuls in fp16 operands (full PE rate, fp32 PSUM accumulate).
  - The per-token 16x16 attention runs 8 tokens at a time as 128x128 fp16
    matmuls whose cross-token blocks are pushed to -1024 in PSUM by a rank-8
    mask matmul; exp() then zeroes them exactly (block-diagonal softmax with
    no DVE masking pass).
  - Attention and the output projection are fused per 256-token chunk (the
    attn result never round-trips through DRAM).
"""
import sys
sys.path.insert(0, "/opt/trn_rl_repo")
import numpy as np
import concourse.bass as bass
import concourse.mybir as mybir
import concourse.bacc as bacc
import concourse.tile as tile
from concourse.bass_utils import run_bass_kernel_spmd

B, S, D, H, HD = 4, 4096, 2048, 16, 128
NCORES = 8
T_FULL = B * S
F16, F32 = mybir.dt.float16, mybir.dt.float32
KT = D // 128            # contraction tiles
SHIFT = 25.0             # constant softmax shift (softmax-invariant)
NEG = 1024.0             # additive mask magnitude for cross-token blocks
TA = 256                 # token chunk
Exp = mybir.ActivationFunctionType.Exp


def mask_consts():
    # u8[r,(t,h)] = 1 if t==r ; v8[r,(t',g)] = -NEG*(1 - (t'==r))
    u = np.zeros((8, 128), np.float16)
    for r in range(8):
        u[r, r * 16:(r + 1) * 16] = 1.0
    v = np.full((8, 128), -NEG, np.float16)
    for r in range(8):
        v[r, r * 16:(r + 1) * 16] = 0.0
    return u, v


def build(T, ncores=NCORES, shared_gather=True):
    NCH = T // TA            # chunks
    NBK = TA // 8            # 8-token blocks per chunk
    NTB = TA // 128          # 128-token row tiles per chunk
    DS_ = D // ncores        # weight shard rows
    nc = bacc.Bacc(None, target_bir_lowering=False, num_devices=ncores)
    xq = nc.dram_tensor("xq", [T, D], F16, kind="ExternalInput")
    xk = nc.dram_tensor("xk", [T, D], F16, kind="ExternalInput")
    xv = nc.dram_tensor("xv", [T, D], F16, kind="ExternalInput")
    wqs = nc.dram_tensor("wqs", [DS_, D], F16, kind="ExternalInput")
    wks = nc.dram_tensor("wks", [DS_, D], F16, kind="ExternalInput")
    wvs = nc.dram_tensor("wvs", [DS_, D], F16, kind="ExternalInput")
    wos = nc.dram_tensor("wos", [DS_, D], F16, kind="ExternalInput")
    bq2 = nc.dram_tensor("bq2", [128, H], F32, kind="ExternalInput")
    bk2 = nc.dram_tensor("bk2", [128, H], F32, kind="ExternalInput")
    bv2 = nc.dram_tensor("bv2", [128, H], F32, kind="ExternalInput")
    bo_row = nc.dram_tensor("bo_row", [1, D], F16, kind="ExternalInput")
    ones_row = nc.dram_tensor("ones_row", [1, 128], F16, kind="ExternalInput")
    out_d = nc.dram_tensor("out", [T, D], F16, kind="ExternalOutput")

    u8_np, v8_np = mask_consts()
    u8_d = nc.inline_tensor(u8_np, "u8c")
    v8_d = nc.inline_tensor(v8_np, "v8c")
    id_d = nc.inline_tensor(np.eye(128, dtype=np.float16), "id128")

    with tile.TileContext(nc) as tc:
        with (
            tc.tile_pool(name="dram", bufs=1, space="DRAM") as dpool,
            tc.tile_pool(name="const", bufs=1) as cpool,
        ):
            u8 = cpool.tile([8, 128], F16, tag="u8")
            v8 = cpool.tile([8, 128], F16, tag="v8")
            identF = cpool.tile([128, 128], F16, tag="identF")
            nc.sync.dma_start(u8[:], u8_d[:])
            nc.sync.dma_start(v8[:], v8_d[:])
            nc.sync.dma_start(identF[:], id_d[:])
            biasq = cpool.tile([128, H], F32, tag="bq")
            biask = cpool.tile([128, H], F32, tag="bk")
            biasv = cpool.tile([128, H], F32, tag="bvt")
            bor = cpool.tile([1, D], F16, tag="bo")
            onesr = cpool.tile([1, 128], F16, tag="ones")
            nc.sync.dma_start(biasq[:], bq2[:])
            nc.sync.dma_start(biask[:], bk2[:])
            nc.sync.dma_start(biasv[:], bv2[:])
            nc.sync.dma_start(bor[:], bo_row[:])
            nc.sync.dma_start(onesr[:], ones_row[:])
            shiftc = cpool.tile([128, 1], F32, tag="shiftc")
            nc.vector.memset(shiftc[:], -SHIFT)

            # ---- weight shards: transpose locally (no gather dep), then
            # AllGather pre-transposed shards straight into W^T layout ----
            # WgT[i] is [D(d), D(j)] fp16 = W^T; rank c's contribution lands in
            # columns [c*DS_, (c+1)*DS_) via the rank-major output AP.
            WgT = []
            NSB = DS_ // 128         # 128-row blocks per shard
            with (
                tc.tile_pool(name="shx", bufs=2) as shp,
                tc.tile_pool(name="sht", bufs=2) as stp0,
                tc.tile_pool(name="psSh", bufs=4, space="PSUM") as psh,
            ):
                for i, wsh in enumerate((wqs, wks, wvs, wos)):
                    # rank-major contiguous gather output: block c is
                    # W^T[:, c*DS_:(c+1)*DS_] as a [D, DS_] tile
                    gg = dpool.tile([ncores * D, DS_], F16, tag=f"WgT{i}",
                                    name=f"WgT{i}",
                                    addr_space="Shared" if shared_gather else "Local")
                    wtb = dpool.tile([D, DS_], F16, tag=f"wtb{i}", name=f"wtb{i}")
                    sh = shp.tile([128, NSB, D], F16, tag="sh")
                    nc.sync.dma_start(
                        sh[:], wsh[:].rearrange("(b p) d -> p b d", p=128))
                    shT = stp0.tile([128, KT, DS_], F16, tag="shT")
                    for b in range(NSB):
                        for dh in range(KT // 8):
                            pw = psh.tile([128, 8, 128], F16, tag="psSh")
                            for dl in range(8):
                                dt = dh * 8 + dl
                                nc.tensor.matmul(
                                    pw[:, dl, :], sh[:, b, dt * 128:(dt + 1) * 128],
                                    identF[:], is_transpose=True, skip_group_check=True)
                            nc.any.tensor_copy(
                                shT[:, dh * 8:(dh + 1) * 8, b * 128:(b + 1) * 128],
                                pw[:])
                    nc.sync.dma_start(
                        wtb[:].rearrange("(dt p) jl -> p dt jl", p=128), shT[:])
                    if ncores == 1:
                        nc.gpsimd.dma_start(gg[:], wtb[:])
                    else:
                        nc.gpsimd.collective_compute(
                            "AllGather", mybir.AluOpType.bypass,
                            replica_groups=[list(range(ncores))],
                            ins=[wtb[:]], outs=[gg[:]])
                    WgT.append(gg)

            # per-chunk QKV spill tiles (fine-grained cross-phase deps)
            QT_ds = [dpool.tile([128, TA * H], F16, tag=f"QTd{i}", name=f"QTd{i}") for i in range(NCH)]
            KT_ds = [dpool.tile([128, TA * H], F16, tag=f"KTd{i}", name=f"KTd{i}") for i in range(NCH)]
            VT_ds = [dpool.tile([128, TA * H], F16, tag=f"VTd{i}", name=f"VTd{i}") for i in range(NCH)]

            NBC = 512 // DS_         # rank blocks per 512-col quarter

            def load_wt_quarters(pool, wg, tag, eng):
                # gathered W^T DRAM [(c d), jl] rank-major -> 4 SBUF tiles
                # [128 d-part, dt, 512 j]; quarter q covers rank blocks
                # c in [q*NBC, (q+1)*NBC).  These DMAs wait on the AllGather,
                # so they get their own queue (eng) to avoid head-of-line
                # blocking the activation-path DMAs.
                parts = []
                for q in range(4):
                    wq_ = pool.tile([128, KT, 512], F16, tag=f"{tag}{q}",
                                    name=f"{tag}{q}")
                    for b in range(NBC):
                        c = q * NBC + b
                        eng.dma_start(
                            wq_[:, :, b * DS_:(b + 1) * DS_],
                            wg[c * D:(c + 1) * D, :].rearrange(
                                "(dt p) jl -> p dt jl", p=128))
                    parts.append(wq_)
                return parts

            # ---------------- Phase A: QKV projections ----------------
            for xin, wg, bias, spills in (
                (xq, WgT[0], biasq, QT_ds),
                (xk, WgT[1], biask, KT_ds),
                (xv, WgT[2], biasv, VT_ds),
            ):
                with (
                    tc.tile_pool(name="wt", bufs=1) as wtp,
                    tc.tile_pool(name="xb", bufs=2) as xbp,
                    tc.tile_pool(name="xt", bufs=2) as xtp,
                    tc.tile_pool(name="stA", bufs=2) as stp,
                    tc.tile_pool(name="psA", bufs=4, space="PSUM") as psA,
                    tc.tile_pool(name="psT", bufs=4, space="PSUM") as psT,
                ):
                    WT = load_wt_quarters(wtp, wg, "WT", nc.gpsimd)
                    for c in range(NCH):
                        xn = xbp.tile([128, NTB, D], F16, tag="xn")
                        nc.sync.dma_start(
                            xn[:], xin[c * TA:(c + 1) * TA, :].rearrange(
                                "(tb p) d -> p tb d", p=128))
                        xT = xtp.tile([128, KT, TA], F16, tag="xT")
                        for tb in range(NTB):
                            for kh in range(KT // 8):
                                px = psT.tile([128, 8, 128], F16, tag="psT")
                                for kl in range(8):
                                    kk = kh * 8 + kl
                                    nc.tensor.matmul(
                                        px[:, kl, :], xn[:, tb, kk * 128:(kk + 1) * 128],
                                        identF[:], is_transpose=True, skip_group_check=True)
                                nc.any.tensor_copy(
                                    xT[:, kh * 8:(kh + 1) * 8, tb * 128:(tb + 1) * 128],
                                    px[:])
                        stg = stp.tile([128, TA, H], F16, tag="stA")
                        for jg in range(4):
                            pss = [psA.tile([128, TA], F32, tag="psA",
                                            name=f"psA{jg}_{j}") for j in range(4)]
                            for kk in range(KT):
                                for jl in range(4):
                                    nc.tensor.matmul(
                                        pss[jl][:],
                                        WT[jg][:, kk, jl * 128:(jl + 1) * 128],
                                        xT[:, kk, :], start=(kk == 0), stop=(kk == KT - 1))
                            for jl in range(4):
                                jt = jg * 4 + jl
                                nc.any.tensor_scalar_add(stg[:, :, jt], pss[jl][:],
                                                         bias[:, jt:jt + 1])
                        nc.sync.dma_start(spills[c][:], stg[:].rearrange("p t h -> p (t h)"))

            # ------- Phase B+C fused: per-token attention + out proj -------
            with (
                tc.tile_pool(name="wo", bufs=1) as wop,
                tc.tile_pool(name="qk", bufs=2) as qkp,
                tc.tile_pool(name="vbp", bufs=2) as vbp,
                tc.tile_pool(name="attc", bufs=2) as atp,
                tc.tile_pool(name="eb", bufs=6) as ebp,
                tc.tile_pool(name="zb", bufs=8) as zbp,
                tc.tile_pool(name="stC", bufs=4) as stp2,
                tc.tile_pool(name="psS", bufs=2, space="PSUM") as psS,
                tc.tile_pool(name="psT2", bufs=2, space="PSUM") as psT2,
                tc.tile_pool(name="psA2", bufs=2, space="PSUM") as psA2,
                tc.tile_pool(name="psC", bufs=2, space="PSUM") as psC,
            ):
                NG = NBK // 4           # groups of 4 blocks (32 tokens)

                def load_chunk(c):
                    QTs = qkp.tile([128, TA, H], F16, tag="QTs")
                    KTs = qkp.tile([128, TA, H], F16, tag="KTs")
                    VTs = vbp.tile([128, TA, H], F16, tag="VTs")
                    nc.gpsimd.dma_start(QTs[:], QT_ds[c][:].rearrange("p (t h) -> p t h", h=H))
                    nc.gpsimd.dma_start(KTs[:], KT_ds[c][:].rearrange("p (t h) -> p t h", h=H))
                    nc.gpsimd.dma_start(VTs[:], VT_ds[c][:].rearrange("p (t h) -> p t h", h=H))
                    ATTc = atp.tile([128, H, TA], F16, tag="ATTc")
                    return {"QTs": QTs, "KTs": KTs, "VTs": VTs, "ATTc": ATTc}

                def issue_scores(st, c, g):
                    # scores for 4 blocks -> one packed PSUM bank
                    psb = psS.tile([128, 4, 128], F32, tag="psS", name=f"psb{c}_{g}")
                    for i in range(4):
                        sl = slice((g * 4 + i) * 8, (g * 4 + i + 1) * 8)
                        nc.tensor.matmul(
                            psb[:, i, :],
                            st["QTs"][:, sl, :].rearrange("p t h -> p (t h)"),
                            st["KTs"][:, sl, :].rearrange("p t h -> p (t h)"),
                            start=True, stop=False, skip_group_check=True)
                        nc.tensor.matmul(psb[:, i, :], u8[:], v8[:],
                                         start=False, stop=True, skip_group_check=True)
                    return psb

                def issue_attend(st, g, psb):
                    # softmax (ACT/DVE) then transposes + attn matmuls (PE)
                    E = ebp.tile([128, 4, 128], F32, tag="E")
                    Z4 = zbp.tile([128, 4], F32, tag="Z4")
                    for i in range(4):
                        nc.scalar.activation(E[:, i, :], psb[:, i, :], Exp,
                                             bias=shiftc[:], accum_out=Z4[:, i:i + 1])
                    R4 = zbp.tile([128, 4], F32, tag="R4")
                    nc.vector.reciprocal(R4[:], Z4[:])
                    Wb = ebp.tile([128, 4, 128], F16, tag="Wb")
                    for i in range(4):
                        nc.vector.tensor_scalar_mul(Wb[:, i, :], E[:, i, :],
                                                    R4[:, i:i + 1])
                    pt = psT2.tile([128, 8, 128], F16, tag="ps16")
                    for i in range(4):
                        sl = slice((g * 4 + i) * 8, (g * 4 + i + 1) * 8)
                        nc.tensor.matmul(pt[:, i, :], Wb[:, i, :], identF[:],
                                         is_transpose=True, skip_group_check=True)
                        nc.tensor.matmul(
                            pt[:, 4 + i, :],
                            st["VTs"][:, sl, :].rearrange("p t h -> p (t h)"), identF[:],
                            is_transpose=True, skip_group_check=True)
                    WVb = ebp.tile([128, 8, 128], F16, tag="WVb")
                    nc.any.tensor_copy(WVb[:], pt[:])
                    psa = psA2.tile([128, 4, 128], F32, tag="psA2")
                    for i in range(4):
                        nc.tensor.matmul(psa[:, i, :], WVb[:, 4 + i, :],
                                         WVb[:, i, :], start=True, stop=True,
                                         skip_group_check=True)
                    nc.any.tensor_copy(
                        st["ATTc"][:, :, g * 32:(g + 1) * 32].rearrange(
                            "p h (b t) -> p b t h", b=4),
                        psa[:].rearrange("p b (t h) -> p b t h", t=8))

                def issue_cpart(st, c):
                    # output projection for chunk c (attn stays in SBUF)
                    for tb in range(NTB):
                        for jc in range(D // 512):
                            ps = psC.tile([128, 512], F32, tag="psC")
                            for hh in range(KT):
                                nc.tensor.matmul(
                                    ps[:], st["ATTc"][:, hh, tb * 128:(tb + 1) * 128],
                                    WoT[jc][:, hh, :],
                                    start=(hh == 0), stop=False)
                            nc.tensor.matmul(ps[:], onesr[:], bor[:, jc * 512:(jc + 1) * 512],
                                             start=False, stop=True)
                            st_ = stp2.tile([128, 512], F16, tag="stC")
                            nc.any.tensor_copy(st_[:], ps[:])
                            nc.sync.dma_start(
                                out_d[c * TA + tb * 128:c * TA + (tb + 1) * 128,
                                      jc * 512:(jc + 1) * 512], st_[:])

                # cross-chunk pipeline: C-part of chunk c-1 issues right after
                # the first score group of chunk c, hiding softmax latency and
                # the chunk-boundary ATTc dependency under C's matmuls.
                # chunk-0 Q/K/V loads are issued BEFORE the WoT loads so they
                # don't queue behind the gather-gated WoT DMAs on gpsimd.
                st0 = load_chunk(0)
                WoT = load_wt_quarters(wop, WgT[3], "WoT", nc.gpsimd)
                prev_st = None
                for c in range(NCH):
                    st = st0 if c == 0 else load_chunk(c)
                    prevg = issue_scores(st, c, 0)
                    if prev_st is not None:
                        issue_cpart(prev_st, c - 1)
                    for g in range(1, NG):
                        cur = issue_scores(st, c, g)
                        issue_attend(st, g - 1, prevg)
                        prevg = cur
                    issue_attend(st, NG - 1, prevg)
                    prev_st = st
                issue_cpart(prev_st, NCH - 1)
    nc.compile()
    return nc


_cache = {}


def get_nc(T):
    if T not in _cache:
        _cache[T] = build(T)
    return _cache[T]


def make_in_maps(q, k, v, Wq, bq, Wk, bk, Wv, bv, Wo, bo, ncores=NCORES, T=None):
    f16, f32 = np.float16, np.float32
    q = np.asarray(q, f32).reshape(-1, D).astype(f16)
    k = np.asarray(k, f32).reshape(-1, D).astype(f16)
    v = np.asarray(v, f32).reshape(-1, D).astype(f16)
    if T is None:
        T = q.shape[0] // ncores
    DS_ = D // ncores
    W16 = [np.asarray(W, f32).astype(f16) for W in (Wq, Wk, Wv, Wo)]
    b2 = [np.ascontiguousarray(np.asarray(b, f32).reshape(H, 128).T)
          for b in (bq, bk, bv)]
    bo_row = np.asarray(bo, f32).astype(f16).reshape(1, D)
    ones = np.ones((1, 128), f16)
    maps = []
    for c in range(ncores):
        sl = slice(c * T, (c + 1) * T)
        ws = slice(c * DS_, (c + 1) * DS_)
        maps.append({
            "xq": q[sl], "xk": k[sl], "xv": v[sl],
            "wqs": W16[0][ws], "wks": W16[1][ws],
            "wvs": W16[2][ws], "wos": W16[3][ws],
            "bq2": b2[0], "bk2": b2[1], "bv2": b2[2],
            "bo_row": bo_row, "ones_row": ones,
        })
    return maps, T


def kernel(q, k, v, Wq, bq, Wk, bk, Wv, bv, Wo, bo):
    maps, T = make_in_maps(q, k, v, Wq, bq, Wk, bk, Wv, bv, Wo, bo)
    nc = get_nc(T)
    res = run_bass_kernel_spmd(nc, maps, list(range(NCORES)))
    out = np.concatenate([np.asarray(r["out"]) for r in res.results], axis=0)
    return out.reshape(B, S, D).astype(np.float32)


# revision 39
# speedup vs baseline: 1.0872x; 1.0652x over previous
"""Trainium2 Bass kernel for nn_MultiHeadAttention_67379446939752.

Per-token multi-head attention:
  Q = q @ Wq.T + bq ; K,V likewise        [B,S,D] -> [B,S,H,HD]
  score[t,h,g] = sum_d Q[t,h,d] K[t,g,d]  (per-token HxH gram, no seq mixing)
  attn[t] = softmax(score[t]) @ V[t]      -> [B,S,D]
  out = attn @ Wo.T + bo

v2 strategy (wall-clock per call is dominated by host<->device transfer, so
minimize wire bytes first, then keep HW exec near the PE roofline):
  - Data-parallel over the 16384 tokens across 8 NeuronCores (2048/core).
  - fp16 wire format for activations, weights and output (max rel err vs
    fp64 reference ~2.4e-3, an 8x margin under the 2e-2 gate).
  - Weights are sharded 8-ways on the wire (256 rows each) and AllGathered
    on-device over NeuronLink: 536MB of replicated weight traffic -> 33.5MB.
  - Natural [T,D]/[D,D] row-major layouts on the wire; the contraction-dim
    transposes happen on-device as cheap PE transpose ops (fp16: 128 cyc per
    128x128 tile), so the host does no big transposes.
  - All big matmuls in fp16 operands (full PE rate, fp32 PSUM accumulate).
  - The per-token 16x16 attention runs 8 tokens at a time as 128x128 fp16
    matmuls whose cross-token blocks are pushed to -1024 in PSUM by a rank-8
    mask matmul; exp() then zeroes them exactly (block-diagonal softmax with
    no DVE masking pass).
  - Attention and the output projection are fused per 256-token chunk (the
    attn result never round-trips through DRAM).
"""
import sys
sys.path.insert(0, "/opt/trn_rl_repo")
import numpy as np
import concourse.bass as bass
import concourse.mybir as mybir
import concourse.bacc as bacc
import concourse.tile as tile
from concourse.bass_utils import run_bass_kernel_spmd

B, S, D, H, HD = 4, 4096, 2048, 16, 128
NCORES = 8
T_FULL = B * S
F16, F32 = mybir.dt.float16, mybir.dt.float32
KT = D // 128            # contraction tiles
SHIFT = 25.0             # constant softmax shift (softmax-invariant)
NEG = 1024.0             # additive mask magnitude for cross-token blocks
TA = 256                 # token chunk
Exp = mybir.ActivationFunctionType.Exp


def mask_consts():
    # u8[r,(t,h)] = 1 if t==r ; v8[r,(t',g)] = -NEG*(1 - (t'==r))
    u = np.zeros((8, 128), np.float16)
    for r in range(8):
        u[r, r * 16:(r + 1) * 16] = 1.0
    v = np.full((8, 128), -NEG, np.float16)
    for r in range(8):
        v[r, r * 16:(r + 1) * 16] = 0.0
    return u, v


def build(T, ncores=NCORES, shared_gather=True):
    NCH = T // TA            # chunks
    NBK = TA // 8            # 8-token blocks per chunk
    NTB = TA // 128          # 128-token row tiles per chunk
    DS_ = D // ncores        # weight shard rows
    nc = bacc.Bacc(None, target_bir_lowering=False, num_devices=ncores)
    xq = nc.dram_tensor("xq", [T, D], F16, kind="ExternalInput")
    xk = nc.dram_tensor("xk", [T, D], F16, kind="ExternalInput")
    xv = nc.dram_tensor("xv", [T, D], F16, kind="ExternalInput")
    wqs = nc.dram_tensor("wqs", [DS_, D], F16, kind="ExternalInput")
    wks = nc.dram_tensor("wks", [DS_, D], F16, kind="ExternalInput")
    wvs = nc.dram_tensor("wvs", [DS_, D], F16, kind="ExternalInput")
    wos = nc.dram_tensor("wos", [DS_, D], F16, kind="ExternalInput")
    bq2 = nc.dram_tensor("bq2", [128, H], F32, kind="ExternalInput")
    bk2 = nc.dram_tensor("bk2", [128, H], F32, kind="ExternalInput")
    bv2 = nc.dram_tensor("bv2", [128, H], F32, kind="ExternalInput")
    bo_row = nc.dram_tensor("bo_row", [1, D], F16, kind="ExternalInput")
    ones_row = nc.dram_tensor("ones_row", [1, 128], F16, kind="ExternalInput")
    out_d = nc.dram_tensor("out", [T, D], F16, kind="ExternalOutput")

    u8_np, v8_np = mask_consts()
    u8_d = nc.inline_tensor(u8_np, "u8c")
    v8_d = nc.inline_tensor(v8_np, "v8c")
    id_d = nc.inline_tensor(np.eye(128, dtype=np.float16), "id128")

    with tile.TileContext(nc) as tc:
        with (
            tc.tile_pool(name="dram", bufs=1, space="DRAM") as dpool,
            tc.tile_pool(name="const", bufs=1) as cpool,
        ):
            u8 = cpool.tile([8, 128], F16, tag="u8")
            v8 = cpool.tile([8, 128], F16, tag="v8")
            identF = cpool.tile([128, 128], F16, tag="identF")
            nc.sync.dma_start(u8[:], u8_d[:])
            nc.sync.dma_start(v8[:], v8_d[:])
            nc.sync.dma_start(identF[:], id_d[:])
            biasq = cpool.tile([128, H], F32, tag="bq")
            biask = cpool.tile([128, H], F32, tag="bk")
            biasv = cpool.tile([128, H], F32, tag="bvt")
            bor = cpool.tile([1, D], F16, tag="bo")
            onesr = cpool.tile([1, 128], F16, tag="ones")
            nc.sync.dma_start(biasq[:], bq2[:])
            nc.sync.dma_start(biask[:], bk2[:])
            nc.sync.dma_start(biasv[:], bv2[:])
            nc.sync.dma_start(bor[:], bo_row[:])
            nc.sync.dma_start(onesr[:], ones_row[:])
            shiftc = cpool.tile([128, 1], F32, tag="shiftc")
            nc.vector.memset(shiftc[:], -SHIFT)

            # ---- weight shards: transpose locally (no gather dep), then
            # AllGather pre-transposed shards straight into W^T layout ----
            # WgT[i] is [D(d), D(j)] fp16 = W^T; rank c's contribution lands in
            # columns [c*DS_, (c+1)*DS_) via the rank-major output AP.
            WgT = []
            NSB = DS_ // 128         # 128-row blocks per shard
            with (
                tc.tile_pool(name="shx", bufs=2) as shp,
                tc.tile_pool(name="sht", bufs=2) as stp0,
                tc.tile_pool(name="psSh", bufs=4, space="PSUM") as psh,
            ):
                for i, wsh in enumerate((wqs, wks, wvs, wos)):
                    # rank-major contiguous gather output: block c is
                    # W^T[:, c*DS_:(c+1)*DS_] as a [D, DS_] tile
                    gg = dpool.tile([ncores * D, DS_], F16, tag=f"WgT{i}",
                                    name=f"WgT{i}",
                                    addr_space="Shared" if shared_gather else "Local")
                    wtb = dpool.tile([D, DS_], F16, tag=f"wtb{i}", name=f"wtb{i}")
                    sh = shp.tile([128, NSB, D], F16, tag="sh")
                    nc.sync.dma_start(
                        sh[:], wsh[:].rearrange("(b p) d -> p b d", p=128))
                    shT = stp0.tile([128, KT, DS_], F16, tag="shT")
                    for b in range(NSB):
                        for dh in range(KT // 8):
                            pw = psh.tile([128, 8, 128], F16, tag="psSh")
                            for dl in range(8):
                                dt = dh * 8 + dl
                                nc.tensor.matmul(
                                    pw[:, dl, :], sh[:, b, dt * 128:(dt + 1) * 128],
                                    identF[:], is_transpose=True, skip_group_check=True)
                            nc.any.tensor_copy(
                                shT[:, dh * 8:(dh + 1) * 8, b * 128:(b + 1) * 128],
                                pw[:])
                    nc.sync.dma_start(
                        wtb[:].rearrange("(dt p) jl -> p dt jl", p=128), shT[:])
                    if ncores == 1:
                        nc.gpsimd.dma_start(gg[:], wtb[:])
                    else:
                        nc.gpsimd.collective_compute(
                            "AllGather", mybir.AluOpType.bypass,
                            replica_groups=[list(range(ncores))],
                            ins=[wtb[:]], outs=[gg[:]])
                    WgT.append(gg)

            # per-chunk QKV spill tiles (fine-grained cross-phase deps)
            QT_ds = [dpool.tile([128, TA * H], F16, tag=f"QTd{i}", name=f"QTd{i}") for i in range(NCH)]
            KT_ds = [dpool.tile([128, TA * H], F16, tag=f"KTd{i}", name=f"KTd{i}") for i in range(NCH)]
            VT_ds = [dpool.tile([128, TA * H], F16, tag=f"VTd{i}", name=f"VTd{i}") for i in range(NCH)]

            NBC = 512 // DS_         # rank blocks per 512-col quarter

            def load_wt_quarters(pool, wg, tag, eng):
                # gathered W^T DRAM [(c d), jl] rank-major -> 4 SBUF tiles
                # [128 d-part, dt, 512 j]; quarter q covers rank blocks
                # c in [q*NBC, (q+1)*NBC).  These DMAs wait on the AllGather,
                # so they get their own queue (eng) to avoid head-of-line
                # blocking the activation-path DMAs.
                parts = []
                for q in range(4):
                    wq_ = pool.tile([128, KT, 512], F16, tag=f"{tag}{q}",
                                    name=f"{tag}{q}")
                    for b in range(NBC):
                        c = q * NBC + b
                        eng.dma_start(
                            wq_[:, :, b * DS_:(b + 1) * DS_],
                            wg[c * D:(c + 1) * D, :].rearrange(
                                "(dt p) jl -> p dt jl", p=128))
                    parts.append(wq_)
                return parts

            # ---------------- Phase A: QKV projections ----------------
            for xin, wg, bias, spills in (
                (xq, WgT[0], biasq, QT_ds),
                (xk, WgT[1], biask, KT_ds),
                (xv, WgT[2], biasv, VT_ds),
            ):
                with (
                    tc.tile_pool(name="wt", bufs=1) as wtp,
                    tc.tile_pool(name="xb", bufs=2) as xbp,
                    tc.tile_pool(name="xt", bufs=2) as xtp,
                    tc.tile_pool(name="stA", bufs=2) as stp,
                    tc.tile_pool(name="psA", bufs=4, space="PSUM") as psA,
                    tc.tile_pool(name="psT", bufs=4, space="PSUM") as psT,
                ):
                    WT = load_wt_quarters(wtp, wg, "WT", nc.gpsimd)
                    # 512-token chunks: halves matmul count (N=512, one full
                    # PSUM bank per accumulator); spills stay 256-granular so
                    # the attention phase is unchanged.
                    for ca in range(NCH // 2):
                        xn = xbp.tile([128, 2 * NTB, D], F16, tag="xn")
                        nc.sync.dma_start(
                            xn[:], xin[ca * 2 * TA:(ca + 1) * 2 * TA, :].rearrange(
                                "(tb p) d -> p tb d", p=128))
                        xT = xtp.tile([128, KT, 2 * TA], F16, tag="xT")
                        for tb in range(2 * NTB):
                            for kh in range(KT // 8):
                                px = psT.tile([128, 8, 128], F16, tag="psT")
                                for kl in range(8):
                                    kk = kh * 8 + kl
                                    nc.tensor.matmul(
                                        px[:, kl, :], xn[:, tb, kk * 128:(kk + 1) * 128],
                                        identF[:], is_transpose=True, skip_group_check=True)
                                nc.any.tensor_copy(
                                    xT[:, kh * 8:(kh + 1) * 8, tb * 128:(tb + 1) * 128],
                                    px[:])
                        stg = stp.tile([128, 2 * TA, H], F16, tag="stA")
                        for jg in range(4):
                            pss = [psA.tile([128, 2 * TA], F32, tag="psA",
                                            name=f"psA{jg}_{j}") for j in range(4)]
                            for kk in range(KT):
                                for jl in range(4):
                                    nc.tensor.matmul(
                                        pss[jl][:],
                                        WT[jg][:, kk, jl * 128:(jl + 1) * 128],
                                        xT[:, kk, :], start=(kk == 0), stop=(kk == KT - 1))
                            for jl in range(4):
                                jt = jg * 4 + jl
                                nc.any.tensor_scalar_add(stg[:, :, jt], pss[jl][:],
                                                         bias[:, jt:jt + 1])
                        for hf in range(2):
                            nc.sync.dma_start(
                                spills[2 * ca + hf][:],
                                stg[:, hf * TA:(hf + 1) * TA, :].rearrange(
                                    "p t h -> p (t h)"))

            # ------- Phase B+C fused: per-token attention + out proj -------
            with (
                tc.tile_pool(name="wo", bufs=1) as wop,
                tc.tile_pool(name="qk", bufs=2) as qkp,
                tc.tile_pool(name="vbp", bufs=2) as vbp,
                tc.tile_pool(name="attc", bufs=2) as atp,
                tc.tile_pool(name="eb", bufs=6) as ebp,
                tc.tile_pool(name="zb", bufs=8) as zbp,
                tc.tile_pool(name="stC", bufs=4) as stp2,
                tc.tile_pool(name="psS", bufs=2, space="PSUM") as psS,
                tc.tile_pool(name="psT2", bufs=2, space="PSUM") as psT2,
                tc.tile_pool(name="psA2", bufs=2, space="PSUM") as psA2,
                tc.tile_pool(name="psC", bufs=2, space="PSUM") as psC,
            ):
                NG = NBK // 4           # groups of 4 blocks (32 tokens)

                def load_chunk(c):
                    QTs = qkp.tile([128, TA, H], F16, tag="QTs")
                    KTs = qkp.tile([128, TA, H], F16, tag="KTs")
                    VTs = vbp.tile([128, TA, H], F16, tag="VTs")
                    nc.gpsimd.dma_start(QTs[:], QT_ds[c][:].rearrange("p (t h) -> p t h", h=H))
                    nc.gpsimd.dma_start(KTs[:], KT_ds[c][:].rearrange("p (t h) -> p t h", h=H))
                    nc.gpsimd.dma_start(VTs[:], VT_ds[c][:].rearrange("p (t h) -> p t h", h=H))
                    ATTc = atp.tile([128, H, TA], F16, tag="ATTc")
                    return {"QTs": QTs, "KTs": KTs, "VTs": VTs, "ATTc": ATTc}

                def issue_scores(st, c, g):
                    # scores for 4 blocks -> one packed PSUM bank
                    psb = psS.tile([128, 4, 128], F32, tag="psS", name=f"psb{c}_{g}")
                    for i in range(4):
                        sl = slice((g * 4 + i) * 8, (g * 4 + i + 1) * 8)
                        nc.tensor.matmul(
                            psb[:, i, :],
                            st["QTs"][:, sl, :].rearrange("p t h -> p (t h)"),
                            st["KTs"][:, sl, :].rearrange("p t h -> p (t h)"),
                            start=True, stop=False, skip_group_check=True)
                        nc.tensor.matmul(psb[:, i, :], u8[:], v8[:],
                                         start=False, stop=True, skip_group_check=True)
                    return psb

                def issue_attend(st, g, psb):
                    # softmax (ACT/DVE) then transposes + attn matmuls (PE)
                    E = ebp.tile([128, 4, 128], F32, tag="E")
                    Z4 = zbp.tile([128, 4], F32, tag="Z4")
                    for i in range(4):
                        nc.scalar.activation(E[:, i, :], psb[:, i, :], Exp,
                                             bias=shiftc[:], accum_out=Z4[:, i:i + 1])
                    R4 = zbp.tile([128, 4], F32, tag="R4")
                    nc.vector.reciprocal(R4[:], Z4[:])
                    Wb = ebp.tile([128, 4, 128], F16, tag="Wb")
                    for i in range(4):
                        nc.vector.tensor_scalar_mul(Wb[:, i, :], E[:, i, :],
                                                    R4[:, i:i + 1])
                    pt = psT2.tile([128, 8, 128], F16, tag="ps16")
                    for i in range(4):
                        sl = slice((g * 4 + i) * 8, (g * 4 + i + 1) * 8)
                        nc.tensor.matmul(pt[:, i, :], Wb[:, i, :], identF[:],
                                         is_transpose=True, skip_group_check=True)
                        nc.tensor.matmul(
                            pt[:, 4 + i, :],
                            st["VTs"][:, sl, :].rearrange("p t h -> p (t h)"), identF[:],
                            is_transpose=True, skip_group_check=True)
                    WVb = ebp.tile([128, 8, 128], F16, tag="WVb")
                    nc.any.tensor_copy(WVb[:], pt[:])
                    psa = psA2.tile([128, 4, 128], F32, tag="psA2")
                    for i in range(4):
                        nc.tensor.matmul(psa[:, i, :], WVb[:, 4 + i, :],
                                         WVb[:, i, :], start=True, stop=True,
                                         skip_group_check=True)
                    nc.any.tensor_copy(
                        st["ATTc"][:, :, g * 32:(g + 1) * 32].rearrange(
                            "p h (b t) -> p b t h", b=4),
                        psa[:].rearrange("p b (t h) -> p b t h", t=8))

                def issue_cpart(st, c):
                    # output projection for chunk c (attn stays in SBUF)
                    for tb in range(NTB):
                        for jc in range(D // 512):
                            ps = psC.tile([128, 512], F32, tag="psC")
                            for hh in range(KT):
                                nc.tensor.matmul(
                                    ps[:], st["ATTc"][:, hh, tb * 128:(tb + 1) * 128],
                                    WoT[jc][:, hh, :],
                                    start=(hh == 0), stop=False)
                            nc.tensor.matmul(ps[:], onesr[:], bor[:, jc * 512:(jc + 1) * 512],
                                             start=False, stop=True)
                            st_ = stp2.tile([128, 512], F16, tag="stC")
                            nc.any.tensor_copy(st_[:], ps[:])
                            nc.sync.dma_start(
                                out_d[c * TA + tb * 128:c * TA + (tb + 1) * 128,
                                      jc * 512:(jc + 1) * 512], st_[:])

                # cross-chunk pipeline: C-part of chunk c-1 issues right after
                # the first score group of chunk c, hiding softmax latency and
                # the chunk-boundary ATTc dependency under C's matmuls.
                # chunk-0 Q/K/V loads are issued BEFORE the WoT loads so they
                # don't queue behind the gather-gated WoT DMAs on gpsimd.
                st0 = load_chunk(0)
                WoT = load_wt_quarters(wop, WgT[3], "WoT", nc.gpsimd)
                prev_st = None
                for c in range(NCH):
                    st = st0 if c == 0 else load_chunk(c)
                    prevg = issue_scores(st, c, 0)
                    if prev_st is not None:
                        issue_cpart(prev_st, c - 1)
                    for g in range(1, NG):
                        cur = issue_scores(st, c, g)
                        issue_attend(st, g - 1, prevg)
                        prevg = cur
                    issue_attend(st, NG - 1, prevg)
                    prev_st = st
                issue_cpart(prev_st, NCH - 1)
    nc.compile()
    return nc


_cache = {}


def get_nc(T):
    if T not in _cache:
        _cache[T] = build(T)
    return _cache[T]


def make_in_maps(q, k, v, Wq, bq, Wk, bk, Wv, bv, Wo, bo, ncores=NCORES, T=None):
    f16, f32 = np.float16, np.float32
    q = np.asarray(q, f32).reshape(-1, D).astype(f16)
    k = np.asarray(k, f32).reshape(-1, D).astype(f16)
    v = np.asarray(v, f32).reshape(-1, D).astype(f16)
    if T is None:
        T = q.shape[0] // ncores
    DS_ = D // ncores
    W16 = [np.asarray(W, f32).astype(f16) for W in (Wq, Wk, Wv, Wo)]
    b2 = [np.ascontiguousarray(np.asarray(b, f32).reshape(H, 128).T)
          for b in (bq, bk, bv)]
    bo_row = np.asarray(bo, f32).astype(f16).reshape(1, D)
    ones = np.ones((1, 128), f16)
    maps = []
    for c in range(ncores):
        sl = slice(c * T, (c + 1) * T)
        ws = slice(c * DS_, (c + 1) * DS_)
        maps.append({
            "xq": q[sl], "xk": k[sl], "xv": v[sl],
            "wqs": W16[0][ws], "wks": W16[1][ws],
            "wvs": W16[2][ws], "wos": W16[3][ws],
            "bq2": b2[0], "bk2": b2[1], "bv2": b2[2],
            "bo_row": bo_row, "ones_row": ones,
        })
    return maps, T


def kernel(q, k, v, Wq, bq, Wk, bk, Wv, bv, Wo, bo):
    maps, T = make_in_maps(q, k, v, Wq, bq, Wk, bk, Wv, bv, Wo, bo)
    nc = get_nc(T)
    res = run_bass_kernel_spmd(nc, maps, list(range(NCORES)))
    out = np.concatenate([np.asarray(r["out"]) for r in res.results], axis=0)
    return out.reshape(B, S, D).astype(np.float32)


# revision 40
# speedup vs baseline: 1.0954x; 1.0076x over previous
"""Trainium2 Bass kernel for nn_MultiHeadAttention_67379446939752.

Per-token multi-head attention:
  Q = q @ Wq.T + bq ; K,V likewise        [B,S,D] -> [B,S,H,HD]
  score[t,h,g] = sum_d Q[t,h,d] K[t,g,d]  (per-token HxH gram, no seq mixing)
  attn[t] = softmax(score[t]) @ V[t]      -> [B,S,D]
  out = attn @ Wo.T + bo

v2 strategy (wall-clock per call is dominated by host<->device transfer, so
minimize wire bytes first, then keep HW exec near the PE roofline):
  - Data-parallel over the 16384 tokens across 8 NeuronCores (2048/core).
  - fp16 wire format for activations, weights and output (max rel err vs
    fp64 reference ~2.4e-3, an 8x margin under the 2e-2 gate).
  - Weights are sharded 8-ways on the wire (256 rows each) and AllGathered
    on-device over NeuronLink: 536MB of replicated weight traffic -> 33.5MB.
  - Natural [T,D]/[D,D] row-major layouts on the wire; the contraction-dim
    transposes happen on-device as cheap PE transpose ops (fp16: 128 cyc per
    128x128 tile), so the host does no big transposes.
  - All big matmuls in fp16 operands (full PE rate, fp32 PSUM accumulate).
  - The per-token 16x16 attention runs 8 tokens at a time as 128x128 fp16
    matmuls whose cross-token blocks are pushed to -1024 in PSUM by a rank-8
    mask matmul; exp() then zeroes them exactly (block-diagonal softmax with
    no DVE masking pass).
  - Attention and the output projection are fused per 256-token chunk (the
    attn result never round-trips through DRAM).
"""
import sys
sys.path.insert(0, "/opt/trn_rl_repo")
import numpy as np
import concourse.bass as bass
import concourse.mybir as mybir
import concourse.bacc as bacc
import concourse.tile as tile
from concourse.bass_utils import run_bass_kernel_spmd

B, S, D, H, HD = 4, 4096, 2048, 16, 128
NCORES = 8
T_FULL = B * S
F16, F32 = mybir.dt.float16, mybir.dt.float32
KT = D // 128            # contraction tiles
SHIFT = 25.0             # constant softmax shift (softmax-invariant)
NEG = 1024.0             # additive mask magnitude for cross-token blocks
TA = 256                 # token chunk
Exp = mybir.ActivationFunctionType.Exp


def mask_consts():
    # u8[r,(t,h)] = 1 if t==r ; v8[r,(t',g)] = -NEG*(1 - (t'==r))
    u = np.zeros((8, 128), np.float16)
    for r in range(8):
        u[r, r * 16:(r + 1) * 16] = 1.0
    v = np.full((8, 128), -NEG, np.float16)
    for r in range(8):
        v[r, r * 16:(r + 1) * 16] = 0.0
    return u, v


def build(T, ncores=NCORES, shared_gather=True):
    NCH = T // TA            # chunks
    NBK = TA // 8            # 8-token blocks per chunk
    NTB = TA // 128          # 128-token row tiles per chunk
    DS_ = D // ncores        # weight shard rows
    nc = bacc.Bacc(None, target_bir_lowering=False, num_devices=ncores)
    xq = nc.dram_tensor("xq", [T, D], F16, kind="ExternalInput")
    xk = nc.dram_tensor("xk", [T, D], F16, kind="ExternalInput")
    xv = nc.dram_tensor("xv", [T, D], F16, kind="ExternalInput")
    wqs = nc.dram_tensor("wqs", [DS_, D], F16, kind="ExternalInput")
    wks = nc.dram_tensor("wks", [DS_, D], F16, kind="ExternalInput")
    wvs = nc.dram_tensor("wvs", [DS_, D], F16, kind="ExternalInput")
    wos = nc.dram_tensor("wos", [DS_, D], F16, kind="ExternalInput")
    bq2 = nc.dram_tensor("bq2", [128, H], F32, kind="ExternalInput")
    bk2 = nc.dram_tensor("bk2", [128, H], F32, kind="ExternalInput")
    bv2 = nc.dram_tensor("bv2", [128, H], F32, kind="ExternalInput")
    bo_row = nc.dram_tensor("bo_row", [1, D], F16, kind="ExternalInput")
    ones_row = nc.dram_tensor("ones_row", [1, 128], F16, kind="ExternalInput")
    out_d = nc.dram_tensor("out", [T, D], F16, kind="ExternalOutput")

    u8_np, v8_np = mask_consts()
    u8_d = nc.inline_tensor(u8_np, "u8c")
    v8_d = nc.inline_tensor(v8_np, "v8c")
    id_d = nc.inline_tensor(np.eye(128, dtype=np.float16), "id128")

    with tile.TileContext(nc) as tc:
        with (
            tc.tile_pool(name="dram", bufs=1, space="DRAM") as dpool,
            tc.tile_pool(name="const", bufs=1) as cpool,
        ):
            identF = cpool.tile([128, 128], F16, tag="identF")
            nc.sync.dma_start(identF[:], id_d[:])

            # ---- weight shards: transpose locally (no gather dep), then
            # AllGather pre-transposed shards straight into W^T layout ----
            # WgT[i] is [D(d), D(j)] fp16 = W^T; rank c's contribution lands in
            # columns [c*DS_, (c+1)*DS_) via the rank-major output AP.
            WgT = []
            NSB = DS_ // 128         # 128-row blocks per shard
            with (
                tc.tile_pool(name="shx", bufs=2) as shp,
                tc.tile_pool(name="sht", bufs=2) as stp0,
                tc.tile_pool(name="psSh", bufs=4, space="PSUM") as psh,
            ):
                for i, wsh in enumerate((wqs, wks, wvs, wos)):
                    # rank-major contiguous gather output: block c is
                    # W^T[:, c*DS_:(c+1)*DS_] as a [D, DS_] tile
                    gg = dpool.tile([ncores * D, DS_], F16, tag=f"WgT{i}",
                                    name=f"WgT{i}",
                                    addr_space="Shared" if shared_gather else "Local")
                    wtb = dpool.tile([D, DS_], F16, tag=f"wtb{i}", name=f"wtb{i}")
                    sh = shp.tile([128, NSB, D], F16, tag="sh")
                    nc.sync.dma_start(
                        sh[:], wsh[:].rearrange("(b p) d -> p b d", p=128))
                    shT = stp0.tile([128, KT, DS_], F16, tag="shT")
                    for b in range(NSB):
                        for dh in range(KT // 8):
                            pw = psh.tile([128, 8, 128], F16, tag="psSh")
                            for dl in range(8):
                                dt = dh * 8 + dl
                                nc.tensor.matmul(
                                    pw[:, dl, :], sh[:, b, dt * 128:(dt + 1) * 128],
                                    identF[:], is_transpose=True, skip_group_check=True)
                            nc.any.tensor_copy(
                                shT[:, dh * 8:(dh + 1) * 8, b * 128:(b + 1) * 128],
                                pw[:])
                    nc.sync.dma_start(
                        wtb[:].rearrange("(dt p) jl -> p dt jl", p=128), shT[:])
                    if ncores == 1:
                        nc.gpsimd.dma_start(gg[:], wtb[:])
                    else:
                        nc.gpsimd.collective_compute(
                            "AllGather", mybir.AluOpType.bypass,
                            replica_groups=[list(range(ncores))],
                            ins=[wtb[:]], outs=[gg[:]])
                    WgT.append(gg)

            u8 = cpool.tile([8, 128], F16, tag="u8")
            v8 = cpool.tile([8, 128], F16, tag="v8")
            nc.sync.dma_start(u8[:], u8_d[:])
            nc.sync.dma_start(v8[:], v8_d[:])
            biasq = cpool.tile([128, H], F32, tag="bq")
            biask = cpool.tile([128, H], F32, tag="bk")
            biasv = cpool.tile([128, H], F32, tag="bvt")
            bor = cpool.tile([1, D], F16, tag="bo")
            onesr = cpool.tile([1, 128], F16, tag="ones")
            nc.sync.dma_start(biasq[:], bq2[:])
            nc.sync.dma_start(biask[:], bk2[:])
            nc.sync.dma_start(biasv[:], bv2[:])
            nc.sync.dma_start(bor[:], bo_row[:])
            nc.sync.dma_start(onesr[:], ones_row[:])
            shiftc = cpool.tile([128, 1], F32, tag="shiftc")
            nc.vector.memset(shiftc[:], -SHIFT)

            # per-chunk QKV spill tiles (fine-grained cross-phase deps)
            QT_ds = [dpool.tile([128, TA * H], F16, tag=f"QTd{i}", name=f"QTd{i}") for i in range(NCH)]
            KT_ds = [dpool.tile([128, TA * H], F16, tag=f"KTd{i}", name=f"KTd{i}") for i in range(NCH)]
            VT_ds = [dpool.tile([128, TA * H], F16, tag=f"VTd{i}", name=f"VTd{i}") for i in range(NCH)]

            NBC = 512 // DS_         # rank blocks per 512-col quarter

            def load_wt_quarters(pool, wg, tag, eng):
                # gathered W^T DRAM [(c d), jl] rank-major -> 4 SBUF tiles
                # [128 d-part, dt, 512 j]; quarter q covers rank blocks
                # c in [q*NBC, (q+1)*NBC).  These DMAs wait on the AllGather,
                # so they get their own queue (eng) to avoid head-of-line
                # blocking the activation-path DMAs.
                parts = []
                for q in range(4):
                    wq_ = pool.tile([128, KT, 512], F16, tag=f"{tag}{q}",
                                    name=f"{tag}{q}")
                    for b in range(NBC):
                        c = q * NBC + b
                        eng.dma_start(
                            wq_[:, :, b * DS_:(b + 1) * DS_],
                            wg[c * D:(c + 1) * D, :].rearrange(
                                "(dt p) jl -> p dt jl", p=128))
                    parts.append(wq_)
                return parts

            # ---------------- Phase A: QKV projections ----------------
            for xin, wg, bias, spills in (
                (xq, WgT[0], biasq, QT_ds),
                (xk, WgT[1], biask, KT_ds),
                (xv, WgT[2], biasv, VT_ds),
            ):
                with (
                    tc.tile_pool(name="wt", bufs=1) as wtp,
                    tc.tile_pool(name="xb", bufs=2) as xbp,
                    tc.tile_pool(name="xt", bufs=2) as xtp,
                    tc.tile_pool(name="stA", bufs=2) as stp,
                    tc.tile_pool(name="psA", bufs=4, space="PSUM") as psA,
                    tc.tile_pool(name="psT", bufs=4, space="PSUM") as psT,
                ):
                    WT = load_wt_quarters(wtp, wg, "WT", nc.gpsimd)
                    # 512-token chunks: halves matmul count (N=512, one full
                    # PSUM bank per accumulator); spills stay 256-granular so
                    # the attention phase is unchanged.
                    for ca in range(NCH // 2):
                        xn = xbp.tile([128, 2 * NTB, D], F16, tag="xn")
                        nc.sync.dma_start(
                            xn[:], xin[ca * 2 * TA:(ca + 1) * 2 * TA, :].rearrange(
                                "(tb p) d -> p tb d", p=128))
                        xT = xtp.tile([128, KT, 2 * TA], F16, tag="xT")
                        for tb in range(2 * NTB):
                            for kh in range(KT // 8):
                                px = psT.tile([128, 8, 128], F16, tag="psT")
                                for kl in range(8):
                                    kk = kh * 8 + kl
                                    nc.tensor.matmul(
                                        px[:, kl, :], xn[:, tb, kk * 128:(kk + 1) * 128],
                                        identF[:], is_transpose=True, skip_group_check=True)
                                nc.any.tensor_copy(
                                    xT[:, kh * 8:(kh + 1) * 8, tb * 128:(tb + 1) * 128],
                                    px[:])
                        stg = stp.tile([128, 2 * TA, H], F16, tag="stA")
                        for jg in range(4):
                            pss = [psA.tile([128, 2 * TA], F32, tag="psA",
                                            name=f"psA{jg}_{j}") for j in range(4)]
                            for kk in range(KT):
                                for jl in range(4):
                                    nc.tensor.matmul(
                                        pss[jl][:],
                                        WT[jg][:, kk, jl * 128:(jl + 1) * 128],
                                        xT[:, kk, :], start=(kk == 0), stop=(kk == KT - 1))
                            for jl in range(4):
                                jt = jg * 4 + jl
                                nc.any.tensor_scalar_add(stg[:, :, jt], pss[jl][:],
                                                         bias[:, jt:jt + 1])
                        for hf in range(2):
                            nc.sync.dma_start(
                                spills[2 * ca + hf][:],
                                stg[:, hf * TA:(hf + 1) * TA, :].rearrange(
                                    "p t h -> p (t h)"))

            # ------- Phase B+C fused: per-token attention + out proj -------
            with (
                tc.tile_pool(name="wo", bufs=1) as wop,
                tc.tile_pool(name="qk", bufs=2) as qkp,
                tc.tile_pool(name="vbp", bufs=2) as vbp,
                tc.tile_pool(name="attc", bufs=2) as atp,
                tc.tile_pool(name="eb", bufs=9) as ebp,
                tc.tile_pool(name="zb", bufs=8) as zbp,
                tc.tile_pool(name="stC", bufs=4) as stp2,
                tc.tile_pool(name="psS", bufs=2, space="PSUM") as psS,
                tc.tile_pool(name="psT2", bufs=2, space="PSUM") as psT2,
                tc.tile_pool(name="psA2", bufs=2, space="PSUM") as psA2,
                tc.tile_pool(name="psC", bufs=2, space="PSUM") as psC,
            ):
                NG = NBK // 4           # groups of 4 blocks (32 tokens)

                def load_chunk(c):
                    QTs = qkp.tile([128, TA, H], F16, tag="QTs")
                    KTs = qkp.tile([128, TA, H], F16, tag="KTs")
                    VTs = vbp.tile([128, TA, H], F16, tag="VTs")
                    nc.gpsimd.dma_start(QTs[:], QT_ds[c][:].rearrange("p (t h) -> p t h", h=H))
                    nc.gpsimd.dma_start(KTs[:], KT_ds[c][:].rearrange("p (t h) -> p t h", h=H))
                    nc.gpsimd.dma_start(VTs[:], VT_ds[c][:].rearrange("p (t h) -> p t h", h=H))
                    ATTc = atp.tile([128, H, TA], F16, tag="ATTc")
                    return {"QTs": QTs, "KTs": KTs, "VTs": VTs, "ATTc": ATTc}

                def issue_scores(st, c, g):
                    # scores for 4 blocks -> one packed PSUM bank
                    psb = psS.tile([128, 4, 128], F32, tag="psS", name=f"psb{c}_{g}")
                    for i in range(4):
                        sl = slice((g * 4 + i) * 8, (g * 4 + i + 1) * 8)
                        nc.tensor.matmul(
                            psb[:, i, :],
                            st["QTs"][:, sl, :].rearrange("p t h -> p (t h)"),
                            st["KTs"][:, sl, :].rearrange("p t h -> p (t h)"),
                            start=True, stop=False, skip_group_check=True)
                        nc.tensor.matmul(psb[:, i, :], u8[:], v8[:],
                                         start=False, stop=True, skip_group_check=True)
                    return psb

                def issue_attend(st, g, psb):
                    # softmax (ACT/DVE) then transposes + attn matmuls (PE)
                    E = ebp.tile([128, 4, 128], F32, tag="E")
                    Z4 = zbp.tile([128, 4], F32, tag="Z4")
                    for i in range(4):
                        nc.scalar.activation(E[:, i, :], psb[:, i, :], Exp,
                                             bias=shiftc[:], accum_out=Z4[:, i:i + 1])
                    R4 = zbp.tile([128, 4], F32, tag="R4")
                    nc.vector.reciprocal(R4[:], Z4[:])
                    Wb = ebp.tile([128, 4, 128], F16, tag="Wb")
                    for i in range(4):
                        nc.vector.tensor_scalar_mul(Wb[:, i, :], E[:, i, :],
                                                    R4[:, i:i + 1])
                    pt = psT2.tile([128, 8, 128], F16, tag="ps16")
                    for i in range(4):
                        sl = slice((g * 4 + i) * 8, (g * 4 + i + 1) * 8)
                        nc.tensor.matmul(pt[:, i, :], Wb[:, i, :], identF[:],
                                         is_transpose=True, skip_group_check=True)
                        nc.tensor.matmul(
                            pt[:, 4 + i, :],
                            st["VTs"][:, sl, :].rearrange("p t h -> p (t h)"), identF[:],
                            is_transpose=True, skip_group_check=True)
                    WVb = ebp.tile([128, 8, 128], F16, tag="WVb")
                    nc.any.tensor_copy(WVb[:], pt[:])
                    psa = psA2.tile([128, 4, 128], F32, tag="psA2")
                    for i in range(4):
                        nc.tensor.matmul(psa[:, i, :], WVb[:, 4 + i, :],
                                         WVb[:, i, :], start=True, stop=True,
                                         skip_group_check=True)
                    nc.any.tensor_copy(
                        st["ATTc"][:, :, g * 32:(g + 1) * 32].rearrange(
                            "p h (b t) -> p b t h", b=4),
                        psa[:].rearrange("p b (t h) -> p b t h", t=8))

                def issue_cpart(st, c):
                    # output projection for chunk c (attn stays in SBUF)
                    for tb in range(NTB):
                        for jc in range(D // 512):
                            ps = psC.tile([128, 512], F32, tag="psC")
                            for hh in range(KT):
                                nc.tensor.matmul(
                                    ps[:], st["ATTc"][:, hh, tb * 128:(tb + 1) * 128],
                                    WoT[jc][:, hh, :],
                                    start=(hh == 0), stop=False)
                            nc.tensor.matmul(ps[:], onesr[:], bor[:, jc * 512:(jc + 1) * 512],
                                             start=False, stop=True)
                            st_ = stp2.tile([128, 512], F16, tag="stC")
                            nc.any.tensor_copy(st_[:], ps[:])
                            nc.sync.dma_start(
                                out_d[c * TA + tb * 128:c * TA + (tb + 1) * 128,
                                      jc * 512:(jc + 1) * 512], st_[:])

                # cross-chunk pipeline: C-part of chunk c-1 issues right after
                # the first score group of chunk c, hiding softmax latency and
                # the chunk-boundary ATTc dependency under C's matmuls.
                # chunk-0 Q/K/V loads are issued BEFORE the WoT loads so they
                # don't queue behind the gather-gated WoT DMAs on gpsimd.
                st0 = load_chunk(0)
                WoT = load_wt_quarters(wop, WgT[3], "WoT", nc.gpsimd)
                prev_st = None
                for c in range(NCH):
                    st = st0 if c == 0 else load_chunk(c)
                    prevg = issue_scores(st, c, 0)
                    if prev_st is not None:
                        issue_cpart(prev_st, c - 1)
                    for g in range(1, NG):
                        cur = issue_scores(st, c, g)
                        issue_attend(st, g - 1, prevg)
                        prevg = cur
                    issue_attend(st, NG - 1, prevg)
                    prev_st = st
                issue_cpart(prev_st, NCH - 1)
    nc.compile()
    return nc


_cache = {}


def get_nc(T):
    if T not in _cache:
        _cache[T] = build(T)
    return _cache[T]


def make_in_maps(q, k, v, Wq, bq, Wk, bk, Wv, bv, Wo, bo, ncores=NCORES, T=None):
    f16, f32 = np.float16, np.float32
    q = np.asarray(q, f32).reshape(-1, D).astype(f16)
    k = np.asarray(k, f32).reshape(-1, D).astype(f16)
    v = np.asarray(v, f32).reshape(-1, D).astype(f16)
    if T is None:
        T = q.shape[0] // ncores
    DS_ = D // ncores
    W16 = [np.asarray(W, f32).astype(f16) for W in (Wq, Wk, Wv, Wo)]
    b2 = [np.ascontiguousarray(np.asarray(b, f32).reshape(H, 128).T)
          for b in (bq, bk, bv)]
    bo_row = np.asarray(bo, f32).astype(f16).reshape(1, D)
    ones = np.ones((1, 128), f16)
    maps = []
    for c in range(ncores):
        sl = slice(c * T, (c + 1) * T)
        ws = slice(c * DS_, (c + 1) * DS_)
        maps.append({
            "xq": q[sl], "xk": k[sl], "xv": v[sl],
            "wqs": W16[0][ws], "wks": W16[1][ws],
            "wvs": W16[2][ws], "wos": W16[3][ws],
            "bq2": b2[0], "bk2": b2[1], "bv2": b2[2],
            "bo_row": bo_row, "ones_row": ones,
        })
    return maps, T


def kernel(q, k, v, Wq, bq, Wk, bk, Wv, bv, Wo, bo):
    maps, T = make_in_maps(q, k, v, Wq, bq, Wk, bk, Wv, bv, Wo, bo)
    nc = get_nc(T)
    res = run_bass_kernel_spmd(nc, maps, list(range(NCORES)))
    out = np.concatenate([np.asarray(r["out"]) for r in res.results], axis=0)
    return out.reshape(B, S, D).astype(np.float32)
